# revision 60
# baseline (speedup 1.0000x reference)
"""Trainium2 Bass kernel for nn_Attention_89172110999574.

Strategy (8 NeuronCores, data parallel — 1 batch element per core):
  - x transposed on-chip via PE; QKV projections as matmuls (fp32r).
  - Scores computed TRANSPOSED (ST[j,i] = k_j . q_i) so that softmax
    reduction rides the attn@V matmul: rhs is exp(ST), lhsT is [v | 1],
    giving the softmax denominator as an extra output row for free.
  - Relative-position bias: the bias matrix is block-Toeplitz, so a
    per-head strip table MS[(g,cj), h, u, ci] = 32*E_h[|u-31-g|, |ci-cj|]
    is precomputed ON HOST and shipped as a DRAM input in fp8e4m3 as a
    (main, residual) pair; every score tile's bias is then added into
    PSUM by ONE DoubleRow identity matmul (0.5 cycles/row) that sums
    main+residual — half the PE cost of the fp32 identity add, ~exact.
  - exp() without max-subtraction (scores are ~N(0,1); safe in fp32),
    gelu deferred to a single phase to avoid ACT table-set switches.
  - fp32r (full-rate fp32 PE mode) for all big matmuls.
"""

import os
import sys

import numpy as np

for _p in ("/opt/trn_rl_repo", "/root/.axon_site/_ro/trn_rl_repo"):
    if os.path.isdir(_p) and _p not in sys.path:
        sys.path.insert(0, _p)

import concourse.bass as bass
import concourse.tile as tile
from concourse import mybir
from concourse.bass_utils import run_bass_kernel_spmd
from concourse.masks import make_identity

N = 1024          # tokens per batch (32*32)
D = 256           # model dim
H = 8             # heads
DK = 32           # head dim (qk)
DV = 64           # head dim (v)
DOUT = 256        # output dim
NCORES = 8
FM = 32           # fmap
SCALE = float(DK) ** -0.5          # 1/sqrt(32)
BN_C = float(1.0 / np.sqrt(1.0 + 1e-5))
F32 = mybir.dt.float32
F32R = mybir.dt.float32r
F8 = mybir.dt.float8e4

MDT = F32R

# ms strip table geometry (free-dim element strides inside the SBUF tile)
MS_U = 66
MS_HSTR = MS_U * 32          # 2112 elements per head
MS_TSTR = H * MS_HSTR        # 16896 elements per (main|residual) plane


def build_nc():
    nc = bass.Bass("TRN2", target_bir_lowering=False, debug=False)

    x = nc.dram_tensor("x", [N, D], F32, kind="ExternalInput").ap()
    wq = nc.dram_tensor("wq", [D, H * DK], F32R, kind="ExternalInput").ap()
    wk = nc.dram_tensor("wk", [D, H * DK], F32R, kind="ExternalInput").ap()
    wv = nc.dram_tensor("wv", [D, H * DV], F32R, kind="ExternalInput").ap()
    wo = nc.dram_tensor("wo", [H * DV, DOUT], F32R, kind="ExternalInput").ap()
    ms8d = nc.dram_tensor("ms8", [128, 2, H, MS_U, 32], F8,
                          kind="ExternalInput").ap()
    bo = nc.dram_tensor("bo", [DOUT], F32, kind="ExternalInput").ap()
    gam = nc.dram_tensor("gam", [DOUT], F32, kind="ExternalInput").ap()
    bet = nc.dram_tensor("bet", [DOUT], F32, kind="ExternalInput").ap()
    out = nc.dram_tensor("out", [N, DOUT], F32, kind="ExternalOutput").ap()

    # scratch DRAM for partition-broadcasting the softmax recip rows
    rrd = nc.dram_tensor("rrd", [H, N], F32R).ap()

    with tile.TileContext(nc) as tc:
        with (
            tc.tile_pool(name="const", bufs=1) as constp,
            tc.tile_pool(name="big", bufs=1) as bigp,
            tc.tile_pool(name="xin", bufs=3) as xinp,
            tc.tile_pool(name="exps", bufs=5) as expp,
            tc.tile_pool(name="small", bufs=2) as smallp,
            tc.tile_pool(name="yout", bufs=3) as youtp,
            tc.tile_pool(name="pss", bufs=2, space="PSUM") as pssp,
            tc.tile_pool(name="psp", bufs=2, space="PSUM") as pspp,
        ):
            # ---------------- phase A: x -> xT --------------------------
            # x tiles are DMA'd first: the serial DMA device must deliver
            # them before anything else so PE can start transposing.
            ident = constp.tile([128, 128], F32)
            make_identity(nc, ident)
            xT = bigp.tile([128, 2, N], MDT)
            # x loaded in two batched strided DMAs (one HWDGE slot each):
            # xa[p, nt, d] = x[128*nt + p, d]
            xa = xinp.tile([128, 8, D], F32, tag="xa", bufs=1)
            for half in range(2):
                eng = nc.scalar if half == 0 else nc.sync
                eng.dma_start(
                    out=xa[:, 4 * half:4 * (half + 1), :],
                    in_=bass.AP(tensor=x.tensor, offset=half * 4 * 128 * D,
                                ap=[[D, 128], [128 * D, 4], [1, D]]),
                )

            # batched weight loads: w_sb[p, kt, c] = w[128*kt + p, c]
            wq_sb = constp.tile([128, 2, 256], MDT)
            wk_sb = constp.tile([128, 2, 256], MDT)
            wv_sb = constp.tile([128, 2, 512], MDT)
            wo_sb = constp.tile([128, 4, 256], MDT)
            for dst_sb, wsrc, nk, eng in (
                    (wq_sb, wq, 2, nc.scalar), (wk_sb, wk, 2, nc.sync),
                    (wv_sb, wv, 2, nc.sync)):
                cols = dst_sb.shape[2]
                eng.dma_start(
                    out=dst_sb,
                    in_=bass.AP(tensor=wsrc.tensor, offset=0,
                                ap=[[cols, 128], [128 * cols, nk], [1, cols]]),
                )

            ms8 = constp.tile([128, 2, H, MS_U, 32], F8)
            nc.scalar.dma_start(out=ms8[:, :, 0:2], in_=ms8d[:, :, 0:2])

            # BN affine rows early on the wire (tiny; consumed ~20us in, and
            # their ACT/DVE ops head-of-line-block those queues if late)
            g2b = constp.tile([128, DOUT], F32)
            b2b = constp.tile([128, DOUT], F32)
            tmpb = constp.tile([128, DOUT], F32)
            nc.scalar.dma_start(
                out=g2b, in_=bass.AP(tensor=gam.tensor, offset=0,
                                     ap=[[0, 128], [1, DOUT]]))
            nc.scalar.dma_start(
                out=b2b, in_=bass.AP(tensor=bet.tensor, offset=0,
                                     ap=[[0, 128], [1, DOUT]]))
            nc.scalar.dma_start(
                out=tmpb, in_=bass.AP(tensor=bo.tensor, offset=0,
                                      ap=[[0, 128], [1, DOUT]]))

            # alternate the two PSUM pools: 4 buffers deep through phase A/B
            def ps_tile(i):
                pool, tag = ((pssp, "st"), (pspp, "po"))[i % 2]
                psf = pool.tile([128, 1024], F32, tag=tag)
                return psf

            for nt in range(8):
                for dt in range(2):
                    pst = ps_tile(2 * nt + dt)
                    nc.tensor.transpose(pst[:, 0:128],
                                        xa[:, nt, 128 * dt:128 * (dt + 1)],
                                        ident)
                    nc.vector.tensor_copy(xT[:, dt, 128 * nt:128 * (nt + 1)],
                                          pst[:, 0:128])

            id8 = constp.tile([128, 128], F8)
            nc.vector.tensor_copy(id8, ident)
            ones_col = constp.tile([1, 64], MDT)
            nc.scalar.activation(ones_col, ident[0:1, 0:64],
                                 mybir.ActivationFunctionType.Copy,
                                 bias=1.0, scale=0.0)

            # ---------------- phase B: QKV proj -------------------------
            qT = bigp.tile([128, 2, N], MDT)
            kT = bigp.tile([128, 2, N], MDT)
            pcnt = 0
            for ic in range(2):
                for dst_sb, w_sb, ceng in ((qT, wq_sb, nc.scalar),
                                           (kT, wk_sb, nc.vector)):
                    for mt in range(2):
                        pcnt += 1
                        ps = ps_tile(pcnt)
                        for kt in range(2):
                            nc.tensor.matmul(
                                ps[:, 0:512],
                                w_sb[:, kt, 128 * mt:128 * (mt + 1)],
                                xT[:, kt, 512 * ic:512 * (ic + 1)],
                                start=(kt == 0), stop=(kt == 1),
                            )
                        if ceng is nc.scalar:
                            ceng.copy(dst_sb[:, mt, 512 * ic:512 * (ic + 1)],
                                      ps[:, 0:512])
                        else:
                            ceng.tensor_copy(dst_sb[:, mt, 512 * ic:512 * (ic + 1)],
                                             ps[:, 0:512])

            # v, augmented with a ones column per head for the softmax
            # denominator: lhsT = [v | 1] -> denominator lands at out row 64.
            va = bigp.tile([128, 8, 8, 65], MDT)
            nc.scalar.activation(va[:, :, :, 64:65],
                                 ident[:, 0:64],
                                 mybir.ActivationFunctionType.Copy,
                                 bias=1.0, scale=0.0)
            for jt in range(8):
                psf = ps_tile(jt)
                ps = psf[:, 0:512]
                for kt in range(2):
                    nc.tensor.matmul(
                        ps,
                        xT[:, kt, 128 * jt:128 * (jt + 1)],
                        wv_sb[:, kt, :],
                        start=(kt == 0), stop=(kt == 1),
                    )
                psr = ps.rearrange("p (h v) -> p h v", v=64)
                if jt % 2 == 0:
                    nc.scalar.copy(va[:, jt, :, 0:64], psr)
                else:
                    nc.vector.tensor_copy(va[:, jt, :, 0:64], psr)

            # remaining bias chunks + wo; g2 = gamma*c ; b2 = bo*g2 + beta
            nc.sync.dma_start(out=ms8[:, :, 2:4], in_=ms8d[:, :, 2:4])
            nc.sync.dma_start(out=ms8[:, :, 4:6], in_=ms8d[:, :, 4:6])
            nc.sync.dma_start(out=ms8[:, :, 6:8], in_=ms8d[:, :, 6:8])
            nc.scalar.dma_start(
                out=wo_sb,
                in_=bass.AP(tensor=wo.tensor, offset=0,
                            ap=[[256, 128], [128 * 256, 4], [1, 256]]),
            )
            nc.scalar.mul(g2b, g2b, BN_C)
            nc.vector.tensor_mul(tmpb, tmpb, g2b)
            nc.vector.tensor_add(b2b, b2b, tmpb)

            # ---------------- phase C: attention ------------------------
            # per (h, jt, ic): kq matmul (512c) + DoubleRow bias identity
            # matmul (256c) into the same PSUM region; exp on ACT; attnV
            # one stage behind on PE.
            id8_pair = bass.AP(tensor=id8.tensor, offset=id8.offset,
                               ap=[id8.ap[0], [0, 2], [1, 128]])
            gT = bigp.tile([128, 4, N], MDT)

            # The 64 (h, jt) tiles run as ONE flat software pipeline with
            # attnV lagging its tile by TWO steps: per step T, PE sees
            # [kq+bias(T), attnV(T-2)]. kq(T) gates exp(T-2) (score slot,
            # 2 buffers) while attnV(T-2) gates the same exp but sits
            # BEHIND kq in queue order, so after each exp only 640ns of
            # kq+bias separates it from the next exp: ACT stays the pacer.
            # po is double-buffered, so normalization is fully off the
            # critical path; its PE broadcast goes two steps later and
            # lands in po's own rows (no extra PSUM slot).
            def normalize_a(h, po, last):
                rows = slice(64 * (h % 2), 64 * (h % 2) + 64)
                kt4 = h // 2
                rr = smallp.tile([1, N], MDT, tag="rr", bufs=2)
                with nc.allow_low_precision(reason="f32r is f32 bits"):
                    # split so the first broadcast can fire half-way in
                    nc.vector.reciprocal(rr[:, 0:512], po[64:65, 0:512])
                    nc.vector.reciprocal(rr[:, 512:1024], po[64:65, 512:1024])
                if last:
                    # tail: ACT is free; stage po into gT across engines so
                    # the final PE broadcast + multiply can run ASAP
                    nc.vector.tensor_copy(gT[rows, kt4, 0:512],
                                          po[0:64, 0:512])
                    nc.scalar.copy(gT[rows, kt4, 512:1024], po[0:64, 512:1024])
                else:
                    # steady state: DMA-bounce the recip row for the
                    # partition broadcast — PE is the phase-C pacer, so
                    # keep it free; DMA/HWDGE are idle here
                    nc.sync.dma_start(out=rrd[h, :].unsqueeze(0), in_=rr)
                return rr

            def normalize_b(h, po, rr, dps=None):
                rows = slice(64 * (h % 2), 64 * (h % 2) + 64)
                kt4 = h // 2
                if dps is None:
                    rrb = smallp.tile([64, N], F32R, tag="rrb", bufs=2)
                    nc.sync.dma_start(
                        out=rrb,
                        in_=bass.AP(tensor=rrd.tensor, offset=h * N,
                                    ap=[[0, 64], [1, N]]),
                    )
                    nc.vector.tensor_mul(gT[rows, kt4, 0:512],
                                         po[0:64, 0:512], rrb[:, 0:512])
                    nc.vector.tensor_mul(gT[rows, kt4, 512:1024],
                                         po[0:64, 512:1024], rrb[:, 512:1024])
                    return
                for ic in range(2):
                    nc.tensor.matmul(
                        dps[0:64, 512 * ic:512 * (ic + 1)],
                        ones_col,
                        rr[:, 512 * ic:512 * (ic + 1)],
                        start=True, stop=True,
                    )
                nc.vector.tensor_mul(gT[rows, kt4, 0:512],
                                     gT[rows, kt4, 0:512], dps[0:64, 0:512])
                nc.vector.tensor_mul(gT[rows, kt4, 512:1024],
                                     gT[rows, kt4, 512:1024],
                                     dps[0:64, 512:1024])

            es_tiles = {}
            pos = {}
            rrs = {}
            for T in range(67):
                # attnV first within the step: it is ready (its exp is 3
                # steps old) and must not sit behind kq's sem-wait in the
                # in-order PE sequencer
                if T >= 3 and T <= 66:
                    hp, jp = divmod(T - 3, 8)
                    esp = es_tiles.pop(T - 3)
                    for ic in range(2):
                        nc.tensor.matmul(
                            pos[hp][0:65, 512 * ic:512 * (ic + 1)],
                            va[:, jp, hp, :],
                            esp[:, 512 * ic:512 * (ic + 1)],
                            start=(jp == 0), stop=(jp == 7),
                        )
                    if jp == 7:
                        rrs[hp] = normalize_a(hp, pos[hp], last=(hp == H - 1))
                if T < 64:
                    h, jt = divmod(T, 8)
                    if jt == 0:
                        po = pspp.tile([128, 1024], F32, tag="po")
                        pos[h] = po
                    mtk = h // 4
                    pb = 32 * (h % 4)
                    ps = pssp.tile([128, 1024], F32, tag="st")
                    for ic in range(2):
                        nc.tensor.matmul(
                            ps[:, 512 * ic:512 * (ic + 1)],
                            kT[pb:pb + 32, mtk, 128 * jt:128 * (jt + 1)],
                            qT[pb:pb + 32, mtk, 512 * ic:512 * (ic + 1)],
                            start=True, stop=False,
                            tile_position=(pb, 0),
                        )
                        u0 = 16 * ic + 31 - 4 * jt
                        nc.tensor.matmul(
                            ps[:, 512 * ic:512 * (ic + 1)],
                            id8_pair,
                            bass.AP(tensor=ms8.tensor,
                                    offset=(ms8.offset + h * MS_HSTR
                                            + u0 * 32),
                                    ap=[ms8.ap[0], [MS_TSTR, 2], [1, 512]]),
                            start=False, stop=True,
                            perf_mode=mybir.MatmulPerfMode.DoubleRow,
                        )
                    es = expp.tile([128, 1024], MDT, tag="es")
                    nc.scalar.activation(es, ps,
                                         mybir.ActivationFunctionType.Exp,
                                         scale=SCALE)
                    es_tiles[T] = es
                if T >= 7:
                    hq, jq = divmod(T - 7, 8)
                    if jq == 7:
                        normalize_b(hq, pos.pop(hq), rrs.pop(hq))
            # warm the Gelu table while the last normalize drains; the last
            # head's broadcast lands in the OTHER po buffer so it does not
            # wait for the staging copies to release its own
            gwarm = smallp.tile([1, 8], F32, tag="gwarm", bufs=1)
            nc.scalar.activation(gwarm, ident[0:1, 0:8],
                                 mybir.ActivationFunctionType.Gelu)
            dps7 = pspp.tile([128, 1024], F32, tag="po")
            normalize_b(H - 1, pos.pop(H - 1), rrs.pop(H - 1), dps=dps7)

            # ------- phase D/E: gelu + out proj + BN, pipelined ---------
            # gelu is applied per 128-column block (all 4 kt chunks of that
            # block in one ACT op) so each out-proj tile can start right
            # after its own gelu, overlapping ACT and PE in the tail.
            yt = youtp.tile([128, 8, DOUT], F32, tag="yt", bufs=1)
            for it in range(8):
                gsl = bass.AP(tensor=gT.tensor,
                              offset=gT.offset + 128 * it,
                              ap=[gT.ap[0], [N, 4], [1, 128]])
                nc.scalar.activation(gsl, gsl,
                                     mybir.ActivationFunctionType.Gelu)
                psf = ps_tile(it)
                ps = psf[:, 0:512]
                for kt in range(4):
                    nc.tensor.matmul(
                        ps[:, 0:256],
                        gT[:, kt, 128 * it:128 * (it + 1)],
                        wo_sb[:, kt, :],
                        start=(kt == 0), stop=(kt == 3),
                    )
                nc.vector.tensor_mul(yt[:, it, :], ps[:, 0:256], g2b)
                nc.vector.tensor_add(yt[:, it, :], yt[:, it, :], b2b)
                if it % 2 == 1:
                    # batched store: out[128*nt + p, :] = yt[p, nt, :]
                    eng = nc.sync if it % 4 == 1 else nc.scalar
                    eng.dma_start(
                        out=bass.AP(tensor=out.tensor,
                                    offset=(it - 1) * 128 * DOUT,
                                    ap=[[DOUT, 128], [128 * DOUT, 2],
                                        [1, DOUT]]),
                        in_=yt[:, it - 1:it + 1, :],
                    )

    _split_excess_waits(nc)
    return nc


def _split_excess_waits(nc):
    """walrus rejects >1 sem-wait per instruction ("Too many sync wait
    commands"); unroll extras into a chain of single-wait same-engine
    NoOps directly before the instruction."""
    ctr = 0
    for fn in nc.m.functions:
        for blk in fn.blocks:
            out = []
            for inst in blk.instructions:
                si = inst.sync_info
                if si is not None and len(si.on_wait) > 1:
                    for w in si.on_wait[:-1]:
                        nop = mybir.InstNoOp(name=f"waitnop-{ctr}")
                        ctr += 1
                        nop.engine = inst.engine
                        nop.sync_info = mybir.SyncInfo(
                            on_wait=[w], on_update=[])
                        out.append(nop)
                    inst.sync_info = mybir.SyncInfo(
                        on_wait=[si.on_wait[-1]], on_update=list(si.on_update))
                out.append(inst)
            blk.instructions = out


def _build_ms8(pos_emb: np.ndarray) -> np.ndarray:
    """Host-precompute the fp8 (main, residual) bias strip table.

    table[(g,cj), t, h, u, ci] approximates 32*E_h[|u-31-g|, |ci-cj|]
    (main + residual), where E = pos_emb.reshape(32, 32, H).
    """
    import ml_dtypes

    E = np.asarray(pos_emb, dtype=np.float32).reshape(32, 32, H)
    T = E.transpose(2, 0, 1)                                   # [h, a, b]
    g = np.arange(4)
    u = np.arange(MS_U)
    a_idx = np.abs(u[None, :] - 31 - g[:, None]).clip(0, 31)   # [4, 66]
    c = np.arange(32)
    b_idx = np.abs(c[None, :] - c[:, None])                    # [cj, ci]
    tmp = T[:, a_idx]                                          # [h, 4, 66, b]
    tab = tmp[:, :, :, b_idx]                                  # [h, 4, 66, cj, ci]
    # -> [(g, cj), h, u, ci]
    arr = np.ascontiguousarray(tab.transpose(1, 3, 0, 2, 4)).reshape(
        4 * 32, H, MS_U, 32) * np.float32(DK)
    main = arr.astype(ml_dtypes.float8_e4m3)
    res = (arr - main.astype(np.float32)).astype(ml_dtypes.float8_e4m3)
    return np.ascontiguousarray(
        np.stack([main, res], axis=1))                         # [128, 2, h, u, ci]


_NC_CACHE = None


def kernel(**inputs) -> np.ndarray:
    global _NC_CACHE
    x = np.ascontiguousarray(inputs["x"], dtype=np.float32)        # (8,32,32,256)
    shared = {
        "wq": np.ascontiguousarray(inputs["Wq"], dtype=np.float32),
        "wk": np.ascontiguousarray(inputs["Wk"], dtype=np.float32),
        "wv": np.ascontiguousarray(inputs["Wv"], dtype=np.float32),
        "wo": np.ascontiguousarray(inputs["Wo"], dtype=np.float32),
        "ms8": _build_ms8(inputs["pos_emb"]),
        "bo": np.ascontiguousarray(inputs["bo"], dtype=np.float32),
        "gam": np.ascontiguousarray(inputs["gamma"], dtype=np.float32),
        "bet": np.ascontiguousarray(inputs["beta"], dtype=np.float32),
    }
    in_maps = []
    for c in range(NCORES):
        m = dict(shared)
        m["x"] = np.ascontiguousarray(x[c].reshape(N, D))
        in_maps.append(m)

    if _NC_CACHE is None:
        _NC_CACHE = build_nc()
    res = run_bass_kernel_spmd(_NC_CACHE, in_maps, core_ids=list(range(NCORES)))
    outs = [res.results[c]["out"].reshape(FM, FM, DOUT) for c in range(NCORES)]
    return np.stack(outs, axis=0)


if __name__ == "__main__":
    build_nc()
    print("build ok")


# revision 62
# speedup vs baseline: 1.0208x; 1.0208x over previous
"""Trainium2 Bass kernel for nn_Attention_89172110999574.

Strategy (8 NeuronCores, data parallel — 1 batch element per core):
  - x transposed on-chip via PE; QKV projections as matmuls (fp32r).
  - Scores computed TRANSPOSED (ST[j,i] = k_j . q_i) so that softmax
    reduction rides the attn@V matmul: rhs is exp(ST), lhsT is [v | 1],
    giving the softmax denominator as an extra output row for free.
  - Relative-position bias: the bias matrix is block-Toeplitz, so a
    per-head strip table MS[(g,cj), h, u, ci] = 32*E_h[|u-31-g|, |ci-cj|]
    is precomputed ON HOST and shipped as a DRAM input in fp8e4m3 as a
    (main, residual) pair; every score tile's bias is then added into
    PSUM by ONE DoubleRow identity matmul (0.5 cycles/row) that sums
    main+residual — half the PE cost of the fp32 identity add, ~exact.
  - exp() without max-subtraction (scores are ~N(0,1); safe in fp32),
    gelu deferred to a single phase to avoid ACT table-set switches.
  - fp32r (full-rate fp32 PE mode) for all big matmuls.
"""

import os
import sys

import numpy as np

for _p in ("/opt/trn_rl_repo", "/root/.axon_site/_ro/trn_rl_repo"):
    if os.path.isdir(_p) and _p not in sys.path:
        sys.path.insert(0, _p)

import concourse.bass as bass
import concourse.tile as tile
from concourse import mybir
from concourse.bass_utils import run_bass_kernel_spmd
from concourse.masks import make_identity

N = 1024          # tokens per batch (32*32)
D = 256           # model dim
H = 8             # heads
DK = 32           # head dim (qk)
DV = 64           # head dim (v)
DOUT = 256        # output dim
NCORES = 8
FM = 32           # fmap
SCALE = float(DK) ** -0.5          # 1/sqrt(32)
BN_C = float(1.0 / np.sqrt(1.0 + 1e-5))
F32 = mybir.dt.float32
F32R = mybir.dt.float32r
F8 = mybir.dt.float8e4

MDT = F32R

# ms strip table geometry (free-dim element strides inside the SBUF tile)
MS_U = 66
MS_HSTR = MS_U * 32          # 2112 elements per head
MS_TSTR = H * MS_HSTR        # 16896 elements per (main|residual) plane


def build_nc():
    nc = bass.Bass("TRN2", target_bir_lowering=False, debug=False)

    x = nc.dram_tensor("x", [N, D], F32, kind="ExternalInput").ap()
    wq = nc.dram_tensor("wq", [D, H * DK], F32R, kind="ExternalInput").ap()
    wk = nc.dram_tensor("wk", [D, H * DK], F32R, kind="ExternalInput").ap()
    wv = nc.dram_tensor("wv", [D, H * DV], F32R, kind="ExternalInput").ap()
    wo = nc.dram_tensor("wo", [H * DV, DOUT], F32R, kind="ExternalInput").ap()
    ms8d = nc.dram_tensor("ms8", [128, 2, H, MS_U, 32], F8,
                          kind="ExternalInput").ap()
    bo = nc.dram_tensor("bo", [DOUT], F32, kind="ExternalInput").ap()
    gam = nc.dram_tensor("gam", [DOUT], F32, kind="ExternalInput").ap()
    bet = nc.dram_tensor("bet", [DOUT], F32, kind="ExternalInput").ap()
    out = nc.dram_tensor("out", [N, DOUT], F32, kind="ExternalOutput").ap()

    # scratch DRAM for partition-broadcasting the softmax recip rows
    rrd = nc.dram_tensor("rrd", [H, N], F32R).ap()

    with tile.TileContext(nc) as tc:
        with (
            tc.tile_pool(name="const", bufs=1) as constp,
            tc.tile_pool(name="big", bufs=1) as bigp,
            tc.tile_pool(name="xin", bufs=3) as xinp,
            tc.tile_pool(name="exps", bufs=5) as expp,
            tc.tile_pool(name="small", bufs=2) as smallp,
            tc.tile_pool(name="yout", bufs=3) as youtp,
            tc.tile_pool(name="pss", bufs=2, space="PSUM") as pssp,
            tc.tile_pool(name="psp", bufs=2, space="PSUM") as pspp,
        ):
            # ---------------- phase A: x -> xT --------------------------
            # x tiles are DMA'd first: the serial DMA device must deliver
            # them before anything else so PE can start transposing.
            ident = constp.tile([128, 128], F32)
            make_identity(nc, ident)
            xT = bigp.tile([128, 2, N], MDT)
            # x loaded in two batched strided DMAs (one HWDGE slot each):
            # xa[p, nt, d] = x[128*nt + p, d]
            xa = xinp.tile([128, 8, D], F32, tag="xa", bufs=1)
            for half in range(2):
                eng = nc.scalar if half == 0 else nc.sync
                eng.dma_start(
                    out=xa[:, 4 * half:4 * (half + 1), :],
                    in_=bass.AP(tensor=x.tensor, offset=half * 4 * 128 * D,
                                ap=[[D, 128], [128 * D, 4], [1, D]]),
                )

            # batched weight loads: w_sb[p, kt, c] = w[128*kt + p, c]
            wq_sb = constp.tile([128, 2, 256], MDT)
            wk_sb = constp.tile([128, 2, 256], MDT)
            wv_sb = constp.tile([128, 2, 512], MDT)
            wo_sb = constp.tile([128, 4, 256], MDT)
            for dst_sb, wsrc, nk, eng in (
                    (wq_sb, wq, 2, nc.scalar), (wk_sb, wk, 2, nc.sync),
                    (wv_sb, wv, 2, nc.sync)):
                cols = dst_sb.shape[2]
                eng.dma_start(
                    out=dst_sb,
                    in_=bass.AP(tensor=wsrc.tensor, offset=0,
                                ap=[[cols, 128], [128 * cols, nk], [1, cols]]),
                )

            ms8 = constp.tile([128, 2, H, MS_U, 32], F8)
            nc.scalar.dma_start(out=ms8[:, :, 0:2], in_=ms8d[:, :, 0:2])

            # BN affine rows early on the wire (tiny; consumed ~20us in, and
            # their ACT/DVE ops head-of-line-block those queues if late)
            g2b = constp.tile([128, DOUT], F32)
            b2b = constp.tile([128, DOUT], F32)
            tmpb = constp.tile([128, DOUT], F32)
            nc.scalar.dma_start(
                out=g2b, in_=bass.AP(tensor=gam.tensor, offset=0,
                                     ap=[[0, 128], [1, DOUT]]))
            nc.scalar.dma_start(
                out=b2b, in_=bass.AP(tensor=bet.tensor, offset=0,
                                     ap=[[0, 128], [1, DOUT]]))
            nc.scalar.dma_start(
                out=tmpb, in_=bass.AP(tensor=bo.tensor, offset=0,
                                      ap=[[0, 128], [1, DOUT]]))

            # alternate the two PSUM pools: 4 buffers deep through phase A/B
            def ps_tile(i):
                pool, tag = ((pssp, "st"), (pspp, "po"))[i % 2]
                psf = pool.tile([128, 1024], F32, tag=tag)
                return psf

            for nt in range(8):
                for dt in range(2):
                    pst = ps_tile(2 * nt + dt)
                    nc.tensor.transpose(pst[:, 0:128],
                                        xa[:, nt, 128 * dt:128 * (dt + 1)],
                                        ident)
                    nc.vector.tensor_copy(xT[:, dt, 128 * nt:128 * (nt + 1)],
                                          pst[:, 0:128])

            id8 = constp.tile([128, 128], F8)
            nc.vector.tensor_copy(id8, ident)
            ones_col = constp.tile([1, 64], MDT)
            nc.scalar.activation(ones_col, ident[0:1, 0:64],
                                 mybir.ActivationFunctionType.Copy,
                                 bias=1.0, scale=0.0)

            # ---------------- phase B: QKV proj -------------------------
            qT = bigp.tile([128, 2, N], MDT)
            kT = bigp.tile([128, 2, N], MDT)
            pcnt = 0
            for ic in range(2):
                for dst_sb, w_sb, ceng in ((qT, wq_sb, nc.scalar),
                                           (kT, wk_sb, nc.vector)):
                    for mt in range(2):
                        pcnt += 1
                        ps = ps_tile(pcnt)
                        for kt in range(2):
                            nc.tensor.matmul(
                                ps[:, 0:512],
                                w_sb[:, kt, 128 * mt:128 * (mt + 1)],
                                xT[:, kt, 512 * ic:512 * (ic + 1)],
                                start=(kt == 0), stop=(kt == 1),
                            )
                        if ceng is nc.scalar:
                            ceng.copy(dst_sb[:, mt, 512 * ic:512 * (ic + 1)],
                                      ps[:, 0:512])
                        else:
                            ceng.tensor_copy(dst_sb[:, mt, 512 * ic:512 * (ic + 1)],
                                             ps[:, 0:512])

            # v, augmented with a ones column per head for the softmax
            # denominator: lhsT = [v | 1] -> denominator lands at out row 64.
            va = bigp.tile([128, 8, 8, 65], MDT)
            nc.scalar.activation(va[:, :, :, 64:65],
                                 ident[:, 0:64],
                                 mybir.ActivationFunctionType.Copy,
                                 bias=1.0, scale=0.0)
            for jt in range(8):
                psf = ps_tile(jt)
                ps = psf[:, 0:512]
                for kt in range(2):
                    nc.tensor.matmul(
                        ps,
                        xT[:, kt, 128 * jt:128 * (jt + 1)],
                        wv_sb[:, kt, :],
                        start=(kt == 0), stop=(kt == 1),
                    )
                psr = ps.rearrange("p (h v) -> p h v", v=64)
                if jt % 2 == 0:
                    nc.scalar.copy(va[:, jt, :, 0:64], psr)
                else:
                    nc.vector.tensor_copy(va[:, jt, :, 0:64], psr)

            # remaining bias chunks + wo; g2 = gamma*c ; b2 = bo*g2 + beta
            nc.sync.dma_start(out=ms8[:, :, 2:4], in_=ms8d[:, :, 2:4])
            nc.sync.dma_start(out=ms8[:, :, 4:6], in_=ms8d[:, :, 4:6])
            nc.sync.dma_start(out=ms8[:, :, 6:8], in_=ms8d[:, :, 6:8])
            nc.scalar.dma_start(
                out=wo_sb,
                in_=bass.AP(tensor=wo.tensor, offset=0,
                            ap=[[256, 128], [128 * 256, 4], [1, 256]]),
            )
            nc.scalar.mul(g2b, g2b, BN_C)
            nc.vector.tensor_mul(tmpb, tmpb, g2b)
            nc.vector.tensor_add(b2b, b2b, tmpb)

            # ---------------- phase C: attention ------------------------
            # per (h, jt, ic): kq matmul (512c) + DoubleRow bias identity
            # matmul (256c) into the same PSUM region; exp on ACT; attnV
            # one stage behind on PE.
            id8_pair = bass.AP(tensor=id8.tensor, offset=id8.offset,
                               ap=[id8.ap[0], [0, 2], [1, 128]])
            gT = bigp.tile([128, 4, N], MDT)

            # The 64 (h, jt) tiles run as ONE flat software pipeline with
            # attnV lagging its tile by TWO steps: per step T, PE sees
            # [kq+bias(T), attnV(T-2)]. kq(T) gates exp(T-2) (score slot,
            # 2 buffers) while attnV(T-2) gates the same exp but sits
            # BEHIND kq in queue order, so after each exp only 640ns of
            # kq+bias separates it from the next exp: ACT stays the pacer.
            # po is double-buffered, so normalization is fully off the
            # critical path; its PE broadcast goes two steps later and
            # lands in po's own rows (no extra PSUM slot).
            def normalize_a(h, po, last):
                rows = slice(64 * (h % 2), 64 * (h % 2) + 64)
                kt4 = h // 2
                rr = smallp.tile([1, N], MDT, tag="rr", bufs=2)
                with nc.allow_low_precision(reason="f32r is f32 bits"):
                    # split so the first broadcast can fire half-way in
                    nc.vector.reciprocal(rr[:, 0:512], po[64:65, 0:512])
                    nc.vector.reciprocal(rr[:, 512:1024], po[64:65, 512:1024])
                nc.vector.tensor_copy(gT[rows, kt4, 0:512], po[0:64, 0:512])
                if last:
                    # tail: ACT is free, split the staging across engines
                    nc.scalar.copy(gT[rows, kt4, 512:1024], po[0:64, 512:1024])
                else:
                    nc.vector.tensor_copy(gT[rows, kt4, 512:1024],
                                          po[0:64, 512:1024])
                return rr

            def normalize_b(h, po, rr):
                rows = slice(64 * (h % 2), 64 * (h % 2) + 64)
                kt4 = h // 2
                for ic in range(2):
                    nc.tensor.matmul(
                        po[0:64, 512 * ic:512 * (ic + 1)],
                        ones_col,
                        rr[:, 512 * ic:512 * (ic + 1)],
                        start=True, stop=True,
                    )
                nc.vector.tensor_mul(gT[rows, kt4, 0:512],
                                     gT[rows, kt4, 0:512], po[0:64, 0:512])
                nc.vector.tensor_mul(gT[rows, kt4, 512:1024],
                                     gT[rows, kt4, 512:1024],
                                     po[0:64, 512:1024])

            es_tiles = {}
            pos = {}
            rrs = {}
            for T in range(67):
                # attnV first within the step: it is ready (its exp is 3
                # steps old) and must not sit behind kq's sem-wait in the
                # in-order PE sequencer
                if T >= 3 and T <= 66:
                    hp, jp = divmod(T - 3, 8)
                    esp = es_tiles.pop(T - 3)
                    for ic in range(2):
                        nc.tensor.matmul(
                            pos[hp][0:65, 512 * ic:512 * (ic + 1)],
                            va[:, jp, hp, :],
                            esp[:, 512 * ic:512 * (ic + 1)],
                            start=(jp == 0), stop=(jp == 7),
                        )
                    if jp == 7:
                        rrs[hp] = normalize_a(hp, pos[hp], last=(hp == H - 1))
                if T < 64:
                    h, jt = divmod(T, 8)
                    if jt == 0:
                        po = pspp.tile([128, 1024], F32, tag="po")
                        pos[h] = po
                    mtk = h // 4
                    pb = 32 * (h % 4)
                    ps = pssp.tile([128, 1024], F32, tag="st")
                    for ic in range(2):
                        nc.tensor.matmul(
                            ps[:, 512 * ic:512 * (ic + 1)],
                            kT[pb:pb + 32, mtk, 128 * jt:128 * (jt + 1)],
                            qT[pb:pb + 32, mtk, 512 * ic:512 * (ic + 1)],
                            start=True, stop=False,
                            tile_position=(pb, 0),
                        )
                        u0 = 16 * ic + 31 - 4 * jt
                        nc.tensor.matmul(
                            ps[:, 512 * ic:512 * (ic + 1)],
                            id8_pair,
                            bass.AP(tensor=ms8.tensor,
                                    offset=(ms8.offset + h * MS_HSTR
                                            + u0 * 32),
                                    ap=[ms8.ap[0], [MS_TSTR, 2], [1, 512]]),
                            start=False, stop=True,
                            perf_mode=mybir.MatmulPerfMode.DoubleRow,
                        )
                    es = expp.tile([128, 1024], MDT, tag="es")
                    nc.scalar.activation(es, ps,
                                         mybir.ActivationFunctionType.Exp,
                                         scale=SCALE)
                    es_tiles[T] = es
                if T >= 7:
                    hq, jq = divmod(T - 7, 8)
                    if jq == 7:
                        normalize_b(hq, pos.pop(hq), rrs.pop(hq))
            # warm the Gelu table while the last normalize drains; the last
            # head's broadcast lands in the OTHER po buffer so it does not
            # wait for the staging copies to release its own
            gwarm = smallp.tile([1, 8], F32, tag="gwarm", bufs=1)
            nc.scalar.activation(gwarm, ident[0:1, 0:8],
                                 mybir.ActivationFunctionType.Gelu)
            dps7 = pspp.tile([128, 1024], F32, tag="po")
            normalize_b(H - 1, dps7, rrs.pop(H - 1))

            # ------- phase D/E: gelu + out proj + BN, pipelined ---------
            # gelu is applied per 128-column block (all 4 kt chunks of that
            # block in one ACT op) so each out-proj tile can start right
            # after its own gelu, overlapping ACT and PE in the tail.
            yt = youtp.tile([128, 8, DOUT], F32, tag="yt", bufs=1)
            for it in range(8):
                gsl = bass.AP(tensor=gT.tensor,
                              offset=gT.offset + 128 * it,
                              ap=[gT.ap[0], [N, 4], [1, 128]])
                nc.scalar.activation(gsl, gsl,
                                     mybir.ActivationFunctionType.Gelu)
                psf = ps_tile(it)
                ps = psf[:, 0:512]
                for kt in range(4):
                    nc.tensor.matmul(
                        ps[:, 0:256],
                        gT[:, kt, 128 * it:128 * (it + 1)],
                        wo_sb[:, kt, :],
                        start=(kt == 0), stop=(kt == 3),
                    )
                nc.vector.tensor_mul(yt[:, it, :], ps[:, 0:256], g2b)
                nc.vector.tensor_add(yt[:, it, :], yt[:, it, :], b2b)
                if it % 2 == 1:
                    # batched store: out[128*nt + p, :] = yt[p, nt, :]
                    eng = nc.sync if it % 4 == 1 else nc.scalar
                    eng.dma_start(
                        out=bass.AP(tensor=out.tensor,
                                    offset=(it - 1) * 128 * DOUT,
                                    ap=[[DOUT, 128], [128 * DOUT, 2],
                                        [1, DOUT]]),
                        in_=yt[:, it - 1:it + 1, :],
                    )

    _split_excess_waits(nc)
    return nc


def _split_excess_waits(nc):
    """walrus rejects >1 sem-wait per instruction ("Too many sync wait
    commands"); unroll extras into a chain of single-wait same-engine
    NoOps directly before the instruction."""
    ctr = 0
    for fn in nc.m.functions:
        for blk in fn.blocks:
            out = []
            for inst in blk.instructions:
                si = inst.sync_info
                if si is not None and len(si.on_wait) > 1:
                    for w in si.on_wait[:-1]:
                        nop = mybir.InstNoOp(name=f"waitnop-{ctr}")
                        ctr += 1
                        nop.engine = inst.engine
                        nop.sync_info = mybir.SyncInfo(
                            on_wait=[w], on_update=[])
                        out.append(nop)
                    inst.sync_info = mybir.SyncInfo(
                        on_wait=[si.on_wait[-1]], on_update=list(si.on_update))
                out.append(inst)
            blk.instructions = out


def _build_ms8(pos_emb: np.ndarray) -> np.ndarray:
    """Host-precompute the fp8 (main, residual) bias strip table.

    table[(g,cj), t, h, u, ci] approximates 32*E_h[|u-31-g|, |ci-cj|]
    (main + residual), where E = pos_emb.reshape(32, 32, H).
    """
    import ml_dtypes

    E = np.asarray(pos_emb, dtype=np.float32).reshape(32, 32, H)
    T = E.transpose(2, 0, 1)                                   # [h, a, b]
    g = np.arange(4)
    u = np.arange(MS_U)
    a_idx = np.abs(u[None, :] - 31 - g[:, None]).clip(0, 31)   # [4, 66]
    c = np.arange(32)
    b_idx = np.abs(c[None, :] - c[:, None])                    # [cj, ci]
    tmp = T[:, a_idx]                                          # [h, 4, 66, b]
    tab = tmp[:, :, :, b_idx]                                  # [h, 4, 66, cj, ci]
    # -> [(g, cj), h, u, ci]
    arr = np.ascontiguousarray(tab.transpose(1, 3, 0, 2, 4)).reshape(
        4 * 32, H, MS_U, 32) * np.float32(DK)
    main = arr.astype(ml_dtypes.float8_e4m3)
    res = (arr - main.astype(np.float32)).astype(ml_dtypes.float8_e4m3)
    return np.ascontiguousarray(
        np.stack([main, res], axis=1))                         # [128, 2, h, u, ci]


_NC_CACHE = None


def kernel(**inputs) -> np.ndarray:
    global _NC_CACHE
    x = np.ascontiguousarray(inputs["x"], dtype=np.float32)        # (8,32,32,256)
    shared = {
        "wq": np.ascontiguousarray(inputs["Wq"], dtype=np.float32),
        "wk": np.ascontiguousarray(inputs["Wk"], dtype=np.float32),
        "wv": np.ascontiguousarray(inputs["Wv"], dtype=np.float32),
        "wo": np.ascontiguousarray(inputs["Wo"], dtype=np.float32),
        "ms8": _build_ms8(inputs["pos_emb"]),
        "bo": np.ascontiguousarray(inputs["bo"], dtype=np.float32),
        "gam": np.ascontiguousarray(inputs["gamma"], dtype=np.float32),
        "bet": np.ascontiguousarray(inputs["beta"], dtype=np.float32),
    }
    in_maps = []
    for c in range(NCORES):
        m = dict(shared)
        m["x"] = np.ascontiguousarray(x[c].reshape(N, D))
        in_maps.append(m)

    if _NC_CACHE is None:
        _NC_CACHE = build_nc()
    res = run_bass_kernel_spmd(_NC_CACHE, in_maps, core_ids=list(range(NCORES)))
    outs = [res.results[c]["out"].reshape(FM, FM, DOUT) for c in range(NCORES)]
    return np.stack(outs, axis=0)


if __name__ == "__main__":
    build_nc()
    print("build ok")
